# revision 9
# baseline (speedup 1.0000x reference)
"""VQ codebook nearest-code search (AudioLDM2 DDCM), 8-way sharded on Trainium2.

Strategy (per spec sharding_hint): shard the 1024-entry codebook across the
8 NeuronCores (128 codes each). Each core computes its partial score matrix
scores[b, k] = <x_b, c_k> for its 128 codes via PE matmuls. fp32 has no DMA
transpose, so each 128x128 codebook tile is transposed on the PE (transpose-
mode matmul into PSUM) and copied back to SBUF by the DVE in groups of 4
(one full PSUM bank per copy). Latents are replicated and pre-transposed on
host into the [128, 250*32] on-chip layout so their DMA is contiguous.

Raw-bass implementation (no Tile): each engine gets an explicit instruction
stream with hand-placed semaphores, at most one sync-wait per instruction
(this walrus build rejects instructions carrying two waits). Pipelining:
PE runs one transpose-group (4 chunks) ahead of the matmuls; DVE copies lag
one group behind the transposes; SP streams codebook tiles (1.6 MB each)
and latent pieces round-robin through 3 buffers.

The tiny O(B*K) epilogue (argmin over (distance, index), gather, usage
scatter-add) is combined on host, exactly mirroring the reference formula:
d2 = |x|^2 + |c|^2 - 2<x,c>, distances = sqrt(max(d2, 0)).
"""

import numpy as np

import concourse.bass as bass
import concourse.mybir as mybir
from concourse.bass_utils import run_bass_kernel_spmd

B = 32              # batch
K = 1024            # codebook size
D = 32000           # flattened latent dim (8*250*16)
NCORES = 8
KSH = K // NCORES   # 128 codes per core
P = 128             # partitions
DCHUNKS = D // P    # 250 contraction chunks of 128
CSUB = 25           # d-subchunks per codebook DMA tile -> [128, 25*128] = 1.6 MB
NTILE = DCHUNKS // CSUB  # 10 codebook DMA tiles
GRP = 4             # transposes per DVE copy group (fills one PSUM bank)
NGRP = (DCHUNKS + GRP - 1) // GRP  # 63 (62 full groups + remainder of 2)
NBUF = 3            # codebook tile buffers
NCT = 3             # transposed-group SBUF buffers
XPIECES = 5         # latent DMA pieces ([128, 50*32] = 800 KB each)
XCH = DCHUNKS // XPIECES  # chunks covered per latent piece
F32 = mybir.dt.float32

_CACHED_NC = None


def _build():
    """One-core bass program (SPMD across the 8 cores).

    Inputs:  xt [128, 250*32] fp32  - latents transposed, chunk-major layout
             cb [128, 32000] fp32   - this core's codebook shard (natural)
    Output:  scores [32, 128] fp32  - x @ cb_shard.T
    """
    nc = bass.Bass()
    xt = nc.dram_tensor("xt", [P, DCHUNKS * B], F32, kind="ExternalInput")
    cb = nc.dram_tensor("cb", [KSH, D], F32, kind="ExternalInput")
    out_s = nc.dram_tensor("scores", [B, KSH], F32, kind="ExternalOutput")

    from contextlib import ExitStack
    with ExitStack() as ctx:
        ident = ctx.enter_context(nc.sbuf_tensor("ident", [P, P], F32))
        xt_sb = ctx.enter_context(nc.sbuf_tensor("xt_sb", [P, DCHUNKS * B], F32))
        c_buf0 = ctx.enter_context(nc.sbuf_tensor("c_buf0", [P, CSUB * P], F32))
        c_buf1 = ctx.enter_context(nc.sbuf_tensor("c_buf1", [P, CSUB * P], F32))
        c_buf2 = ctx.enter_context(nc.sbuf_tensor("c_buf2", [P, CSUB * P], F32))
        ct_buf0 = ctx.enter_context(nc.sbuf_tensor("ct_buf0", [P, GRP * P], F32))
        ct_buf1 = ctx.enter_context(nc.sbuf_tensor("ct_buf1", [P, GRP * P], F32))
        ct_buf2 = ctx.enter_context(nc.sbuf_tensor("ct_buf2", [P, GRP * P], F32))
        out_sb = ctx.enter_context(nc.sbuf_tensor("out_sb", [B, KSH], F32))
        pt_a = ctx.enter_context(nc.psum_tensor("pt_a", [P, GRP * P], F32))
        pt_b = ctx.enter_context(nc.psum_tensor("pt_b", [P, GRP * P], F32))
        spsum = ctx.enter_context(nc.psum_tensor("spsum", [B, KSH], F32))
        s_pool = ctx.enter_context(nc.semaphore("s_pool"))
        s_xps = [ctx.enter_context(nc.semaphore(f"s_xp{q}")) for q in range(XPIECES)]
        s_cbs = [ctx.enter_context(nc.semaphore(f"s_cb{i}")) for i in range(NBUF)]
        s_out = ctx.enter_context(nc.semaphore("s_out"))
        s_pe = ctx.enter_context(nc.semaphore("s_pe"))
        s_cp = ctx.enter_context(nc.semaphore("s_cp"))
        block = ctx.enter_context(nc.Block())
        c_bufs = [c_buf0, c_buf1, c_buf2]
        ct_bufs = [ct_buf0, ct_buf1, ct_buf2]
        pt_bufs = [pt_a, pt_b]

        # PE instruction ordinals (s_pe value after each op completes),
        # recorded during PE emission and consumed by the DVE/SP streams.
        pe_ord = {"n": 0}
        t_idx: dict[int, int] = {}
        m_idx: dict[int, int] = {}

        @block.gpsimd
        def _(gpsimd):
            # gpsimd ops can race each other (8 Q7 cores) - chain via sem
            nc.gpsimd.memset(ident[:], 0.0).then_inc(s_pool)
            gpsimd.wait_ge(s_pool, 1)
            nc.gpsimd.affine_select(
                out=ident[:],
                in_=ident[:],
                compare_op=mybir.AluOpType.not_equal,
                fill=1.0,
                base=0,
                pattern=[[-1, P]],
                channel_multiplier=1,
            ).then_inc(s_pool)

        @block.tensor
        def _(tensor):
            def emit_transpose(j):
                ci = j // CSUB
                if j % CSUB == 0:
                    # new codebook tile: wait for its DMA (per-buffer sem)
                    tensor.wait_ge(s_cbs[ci % NBUF], 16 * (ci // NBUF + 1))
                g = j // GRP
                slot = j % GRP
                nc.tensor.transpose(
                    pt_bufs[g % 2][:, slot * P : (slot + 1) * P],
                    c_bufs[ci % NBUF][:, (j % CSUB) * P : ((j % CSUB) + 1) * P],
                    ident[:],
                ).then_inc(s_pe)
                pe_ord["n"] += 1
                t_idx[j] = pe_ord["n"]

            def emit_matmul(j):
                if j % XCH == 0:
                    # next latent piece must be resident
                    tensor.wait_ge(s_xps[j // XCH], 16)
                g = j // GRP
                slot = j % GRP
                nc.tensor.matmul(
                    spsum[:B, :KSH],
                    xt_sb[:, j * B : (j + 1) * B],
                    ct_bufs[g % NCT][:, slot * P : (slot + 1) * P],
                    start=(j == 0),
                    stop=(j == DCHUNKS - 1),
                    skip_group_check=True,
                ).then_inc(s_pe)
                pe_ord["n"] += 1
                m_idx[j] = pe_ord["n"]

            tensor.wait_ge(s_pool, 2)  # identity ready
            for j in range(GRP):  # group 0 transposes
                emit_transpose(j)
            for g in range(NGRP):
                if g + 1 < NGRP:  # run one transpose-group ahead
                    for j in range(GRP * (g + 1), min(GRP * (g + 2), DCHUNKS)):
                        emit_transpose(j)
                tensor.wait_ge(s_cp, g + 1)  # group g copied to SBUF
                for j in range(GRP * g, min(GRP * (g + 1), DCHUNKS)):
                    emit_matmul(j)

        @block.vector
        def _(vector):
            for g in range(NGRP):
                hi = min(GRP * (g + 1), DCHUNKS)
                width = (hi - GRP * g) * P
                vector.wait_ge(s_pe, t_idx[hi - 1])  # group's transposes done
                nc.vector.tensor_copy(
                    out=ct_bufs[g % NCT][:, :width], in_=pt_bufs[g % 2][:, :width]
                ).then_inc(s_cp)
            vector.wait_ge(s_pe, m_idx[DCHUNKS - 1])  # accumulation finished
            nc.vector.tensor_copy(out=out_sb[:], in_=spsum[:B, :KSH]).then_inc(s_cp)

        @block.sync
        def _(sync):
            # Interleave codebook tiles and latent pieces so the PE can start
            # on transposes quickly (HWDGE DMAs complete in FIFO order).
            xq = 0
            for ci in range(NTILE):
                if ci >= NBUF:
                    # buffer free once the last transpose reading it retired
                    sync.wait_ge(s_pe, t_idx[CSUB * (ci - NBUF) + CSUB - 1])
                sync.dma_start(
                    out=c_bufs[ci % NBUF][:],
                    in_=cb[:, ci * CSUB * P : (ci + 1) * CSUB * P],
                ).then_inc(s_cbs[ci % NBUF], 16)
                if xq < XPIECES:
                    sync.dma_start(
                        out=xt_sb[:, xq * XCH * B : (xq + 1) * XCH * B],
                        in_=xt[:, xq * XCH * B : (xq + 1) * XCH * B],
                    ).then_inc(s_xps[xq], 16)
                    xq += 1
            sync.wait_ge(s_cp, NGRP + 1)  # scores landed in SBUF
            sync.dma_start(out=out_s[:], in_=out_sb[:]).then_inc(s_out, 16)

    return nc


def _get_nc():
    global _CACHED_NC
    if _CACHED_NC is None:
        _CACHED_NC = _build()
    return _CACHED_NC


def _device_scores(x, c, trace=False):
    """Run the sharded device kernel. x: [B, D] f32, c: [K, D] f32.

    Returns (scores [B, K] f32, BassKernelResults)."""
    # Pre-arrange latents to the on-chip layout: xt_sb[p, j*B + b] = x[b, j*128 + p]
    xt = np.ascontiguousarray(
        x.T.reshape(DCHUNKS, P, B).transpose(1, 0, 2).reshape(P, DCHUNKS * B)
    )
    in_maps = [
        {"xt": xt, "cb": np.ascontiguousarray(c[i * KSH : (i + 1) * KSH])}
        for i in range(NCORES)
    ]
    res = run_bass_kernel_spmd(
        _get_nc(), in_maps, core_ids=list(range(NCORES)), trace=trace
    )
    scores = np.concatenate(
        [res.results[i]["scores"] for i in range(NCORES)], axis=1
    )
    return scores, res


def kernel(latents, codebook, usage_count):
    latents = np.asarray(latents, dtype=np.float32)
    codebook = np.asarray(codebook, dtype=np.float32)
    usage_count = np.asarray(usage_count, dtype=np.float32)

    x = latents.reshape(B, D)
    c = codebook.reshape(K, D)

    scores, _ = _device_scores(x, c)

    # Tiny epilogue on host, mirroring the reference formula in fp32.
    x2 = np.sum(x * x, axis=1, keepdims=True, dtype=np.float32)   # [B, 1]
    c2 = np.sum(c * c, axis=1, dtype=np.float32)                  # [K]
    d2 = x2 + c2[None, :] - 2.0 * scores
    distances = np.sqrt(np.maximum(d2, 0.0), dtype=np.float32)    # [B, K]
    indices64 = np.argmin(distances, axis=1)
    indices = indices64.astype(np.int32)
    min_distances = np.take_along_axis(
        distances, indices64[:, None], axis=1
    )[:, 0]
    quantized = codebook[indices64]                               # [B, 8, 250, 16]
    new_usage = usage_count.copy()
    np.add.at(new_usage, indices64, np.float32(1.0))
    return indices, quantized, min_distances, new_usage
